# revision 28
# baseline (speedup 1.0000x reference)
"""Trainium2 Bass kernel for a dense transformer block, sharded over 8 NeuronCores.

Sharding: core c handles batch b=c//2 and half hf=c%2 of that batch's 2048
tokens ("own" tokens). K/V are computed for the full 2048-token batch on both
cores of a pair, so no collectives are needed.

v2: fp16 matmul path (fp32 PSUM accumulation), everything SBUF-resident (no
DRAM round-trip for h), each weight matrix streamed exactly once, exp done in
[128,1024] chunks to amortize ACT overhead.
"""

import numpy as np

from contextlib import ExitStack

import concourse.bass as bass
import concourse.bacc as bacc
import concourse.tile as tile
import concourse.mybir as mybir

F32 = mybir.dt.float32
F32R = mybir.dt.float32r
F16 = mybir.dt.float16
AF = mybir.ActivationFunctionType
OP = mybir.AluOpType

EPS = 1e-5

PHASE_MARKS = []


def _mark(nc, name):
    PHASE_MARKS.append((name, int(nc.get_next_instruction_name()[2:])))


class Cfg:
    def __init__(self, E=1024, H=16, MLP=4096, T_OWN=1024, T_FULL=2048, repeat=1,
                 skip=()):
        self.E, self.H, self.MLP = E, H, MLP
        self.T_OWN, self.T_FULL = T_OWN, T_FULL
        self.D = E // H
        self.NE = E // 128          # feature tiles
        self.NM = MLP // 128        # mlp feature tiles
        self.NQB = T_OWN // 512     # own-token 512-blocks
        self.NFB = T_FULL // 512    # full-token 512-blocks
        self.NTK = T_FULL // 128    # full-token 128-blocks (k positions)
        self.G = 2                  # head groups
        self.HPG = H // self.G      # heads per group
        self.NP_G = self.HPG // 2   # head-pairs per group
        self.repeat = repeat
        self.skip = frozenset(skip)


def build(cfg: Cfg):
    E, MLP, T_OWN, T_FULL = cfg.E, cfg.MLP, cfg.T_OWN, cfg.T_FULL

    nc = bacc.Bacc("TRN2", target_bir_lowering=False, debug=False)

    d = {}
    d["xT"] = nc.dram_tensor("xT", [E, T_FULL], F16, kind="ExternalInput")
    d["qkvT"] = nc.dram_tensor("qkvT", [E, 3 * E], F16, kind="ExternalInput")
    d["fcT"] = nc.dram_tensor("fcT", [E, E], F16, kind="ExternalInput")
    d["w1T"] = nc.dram_tensor("w1T", [E, MLP], F16, kind="ExternalInput")
    d["w2T"] = nc.dram_tensor("w2T", [MLP, E], F16, kind="ExternalInput")
    d["ln1"] = nc.dram_tensor("ln1", [2, E], F32, kind="ExternalInput")
    d["ln2"] = nc.dram_tensor("ln2", [2, E], F32, kind="ExternalInput")
    d["fcb"] = nc.dram_tensor("fcb", [E], F32, kind="ExternalInput")
    d["b1"] = nc.dram_tensor("b1", [MLP], F32, kind="ExternalInput")
    d["b2"] = nc.dram_tensor("b2", [E], F32, kind="ExternalInput")
    d["ones"] = nc.dram_tensor("ones", [T_FULL], F32, kind="ExternalInput")
    d["out"] = nc.dram_tensor("out", [E, T_OWN], F32, kind="ExternalOutput")

    PHASE_MARKS.clear()
    with tile.TileContext(nc) as tc, nc.allow_low_precision(
        reason="fp16 matmul inputs by design"
    ):
        if cfg.repeat == 1:
            _body(nc, tc, cfg, d)
        else:
            with tc.For_i(0, cfg.repeat, 1, hint_engines=(
                    mybir.EngineType.PE, mybir.EngineType.Activation,
                    mybir.EngineType.DVE, mybir.EngineType.SP)):
                _body(nc, tc, cfg, d)
    nc.compile()
    return nc


def _ln_stats(nc, cfg, pools, src_fn, nblk, ones_col, eps_t, srst, snb):
    """Column stats over the feature dim via ones-matmuls (fp16 inputs).

    src_fn(e, tb) -> [128,512] F16 AP; writes rstd into srst[0:1] (f32) and
    -mu*rstd into snb[0:1] (f32)."""
    E, NE = cfg.E, cfg.NE
    sq_pool, st_ps, row_pool = pools
    for tb in range(nblk):
        sl = slice(tb * 512, (tb + 1) * 512)
        s1 = st_ps.tile([1, 512], F32, tag="s1")
        s2 = st_ps.tile([1, 512], F32, tag="s2")
        for e in range(NE):
            src = src_fn(e, tb)
            sq = sq_pool.tile([128, 512], F16, tag="sq")
            nc.vector.tensor_tensor(sq[:], src, src, OP.mult)
            nc.tensor.matmul(s1[:], ones_col[:], src, start=(e == 0), stop=(e == NE - 1))
            nc.tensor.matmul(s2[:], ones_col[:], sq[:], start=(e == 0), stop=(e == NE - 1))
        m_row = row_pool.tile([1, 512], F32, tag="mrow")
        nc.vector.tensor_scalar_mul(m_row[:], s1[:], 1.0 / E)
        v_row = row_pool.tile([1, 512], F32, tag="vrow")
        nc.vector.tensor_scalar_mul(v_row[:], s2[:], 1.0 / E)
        msq = row_pool.tile([1, 512], F32, tag="msq")
        nc.vector.tensor_tensor(msq[:], m_row[:], m_row[:], OP.mult)
        nc.vector.tensor_tensor(v_row[:], v_row[:], msq[:], OP.subtract)
        sd = row_pool.tile([1, 512], F32, tag="sd")
        nc.scalar.activation(sd[:], v_row[:], AF.Sqrt, bias=eps_t[:], scale=1.0)
        nc.vector.reciprocal(srst[0:1, sl], sd[:])
        nc.vector.scalar_tensor_tensor(
            snb[0:1, sl], m_row[:], -1.0, srst[0:1, sl].bitcast(F32),
            op0=OP.mult, op1=OP.mult)


def _ln_apply(nc, map_ps, gb_ap, srst, snb, src_ap, dst_ap, sl):
    """dst(F16) = src * (g x rstd) + (g x (-mu*rstd) + b x 1), all [128,512].

    gb_ap: [2,128] f32 AP (rows g, b) for this feature tile."""
    a_ps = map_ps.tile([128, 512], F32, tag="amap")
    nc.tensor.matmul(a_ps[:], gb_ap[0:1, :],
                     srst[0:1, sl], start=True, stop=True)
    b_ps = map_ps.tile([128, 512], F32, tag="bmap")
    nc.tensor.matmul(b_ps[:], gb_ap[0:2, :],
                     snb[0:2, sl], start=True, stop=True)
    nc.vector.tensor_tensor(dst_ap, src_ap, a_ps[:], OP.mult)
    nc.vector.tensor_tensor(dst_ap, dst_ap, b_ps[:], OP.add)


def _body(nc, tc, cfg, d):
    E, H, MLP, D = cfg.E, cfg.H, cfg.MLP, cfg.D
    NE, NM, NQB, NFB, NTK = cfg.NE, cfg.NM, cfg.NQB, cfg.NFB, cfg.NTK
    T_OWN, T_FULL, G, HPG, NP_G = cfg.T_OWN, cfg.T_FULL, cfg.G, cfg.HPG, cfg.NP_G
    xT, qkvT, fcT, w1T, w2T = d["xT"], d["qkvT"], d["fcT"], d["w1T"], d["w2T"]
    ln1, ln2, fcb, b1, b2 = d["ln1"], d["ln2"], d["fcb"], d["b1"], d["b2"]
    ones, out = d["ones"], d["out"]

    with ExitStack() as ctx:
        consts = ctx.enter_context(tc.tile_pool(name="consts", bufs=1))

        ones_col = consts.tile([128, 1], F16)
        nc.vector.memset(ones_col[:], 1.0)
        ones64 = consts.tile([1, 64], F32R)
        nc.sync.dma_start(ones64[:], ones.ap()[0:64].unsqueeze(0).bitcast(F32R))
        eps_t = consts.tile([1, 1], F32)
        nc.vector.memset(eps_t[:], EPS)
        ln1t = consts.tile([2, E], F32R)
        nc.sync.dma_start(ln1t[:], ln1.ap().bitcast(F32R))
        ln2t = consts.tile([2, E], F32R)
        nc.sync.dma_start(ln2t[:], ln2.ap().bitcast(F32R))
        fcb_t = consts.tile([128, NE], F32)
        nc.sync.dma_start(fcb_t[:], fcb.ap().rearrange("(a p) -> p a", p=128))
        b1_t = consts.tile([128, NM], F32)
        nc.sync.dma_start(b1_t[:], b1.ap().rearrange("(a p) -> p a", p=128))
        b2_t = consts.tile([128, NE], F32)
        nc.sync.dma_start(b2_t[:], b2.ap().rearrange("(a p) -> p a", p=128))

        # ================= Phases 1-3: LN1, QKV, attention =================
        with ExitStack() as p1:
            q_pool = p1.enter_context(tc.tile_pool(name="qp", bufs=NE))
            q_tiles = [q_pool.tile([128, T_OWN], F16, tag="qt", name="qt")
                       for _ in range(NE)]
            k_pool = p1.enter_context(tc.tile_pool(name="kp", bufs=G * NP_G))
            v_pool = p1.enter_context(tc.tile_pool(name="vp", bufs=G * NTK))
            k_tiles, v_tiles = {}, {}

            # --- Phase A: fused LN1 + K/Q projections (per token block) ---
            _mark(nc, "A:ln1")
            hstk = ExitStack()
            h_pool = hstk.enter_context(tc.tile_pool(name="hp", bufs=NE))
            h_tiles = [h_pool.tile([128, T_FULL], F16, tag="ht", name="ht")
                       for _ in range(NE)]
            with ExitStack() as pA:
                xf_pool = pA.enter_context(tc.tile_pool(name="xfp", bufs=2 * NE))
                sq_pool = pA.enter_context(tc.tile_pool(name="sqp", bufs=4))
                st_ps = pA.enter_context(tc.tile_pool(name="stps", bufs=1, space="PSUM"))
                row_pool = pA.enter_context(tc.tile_pool(name="rows", bufs=2))
                map_ps = pA.enter_context(tc.tile_pool(name="mapps", bufs=1, space="PSUM"))
                stat_pool = pA.enter_context(tc.tile_pool(name="statp", bufs=1))
                wq_pool = pA.enter_context(tc.tile_pool(name="wqp", bufs=NE))
                wk_pool = pA.enter_context(tc.tile_pool(name="wkp", bufs=2 * NE))
                acc_ps = pA.enter_context(tc.tile_pool(name="accps", bufs=3, space="PSUM"))
                srst1 = stat_pool.tile([1, T_FULL], F32R, tag="srst1")
                snb1 = stat_pool.tile([2, T_FULL], F32R, tag="snb1")
                nc.sync.dma_start(snb1[1:2, :], ones.ap()[0:T_FULL].unsqueeze(0).bitcast(F32R))

                xts0 = []
                for e in range(NE):
                    t = xf_pool.tile([128, 512], F16, tag="xf", name="xf")
                    eng = nc.sync if e % 2 == 0 else nc.scalar
                    eng.dma_start(t[:], xT.ap()[e * 128:(e + 1) * 128, 0:512])
                    xts0.append(t)
                wq = []
                for e in range(NE):
                    t = wq_pool.tile([128, E], F16, tag="wq", name="wq")
                    nc.scalar.dma_start(t[:], qkvT.ap()[e * 128:(e + 1) * 128, 0:E])
                    wq.append(t)
                wk = {}
                for g in range(G):
                    for e in range(NE):
                        t = wk_pool.tile([128, HPG * D], F16, tag="wk", name="wk")
                        col0 = E + g * HPG * D
                        nc.scalar.dma_start(
                            t[:], qkvT.ap()[e * 128:(e + 1) * 128, col0:col0 + HPG * D])
                        wk[(g, e)] = t
                for g in range(G):
                    for dkt in range(NP_G):
                        k_tiles[(g, dkt)] = k_pool.tile([128, T_FULL], F16,
                                                        tag="kt", name="kt")

                for tb in range(NFB):
                    sl = slice(tb * 512, (tb + 1) * 512)
                    if tb == 0:
                        xts = xts0
                    else:
                        xts = []
                        for e in range(NE):
                            t = xf_pool.tile([128, 512], F16, tag="xf", name="xf")
                            nc.sync.dma_start(t[:], xT.ap()[e * 128:(e + 1) * 128, sl])
                            xts.append(t)
                    _ln_stats(nc, cfg, (sq_pool, st_ps, row_pool),
                              lambda e, _tb: xts[e][:], 1, ones_col, eps_t,
                              srst1[0:1, sl], snb1[0:2, sl])
                    for e in range(NE):
                        _ln_apply(nc, map_ps, ln1t[:, e * 128:(e + 1) * 128],
                                  srst1, snb1, xts[e][:], h_tiles[e][:, sl], sl)
                    # K projections for this token block (both groups)
                    if "kv" not in cfg.skip:
                        for g in range(G):
                            for dkt in range(NP_G):
                                ps = acc_ps.tile([128, 512], F32, tag="acc", name="acc")
                                for e in range(NE):
                                    nc.tensor.matmul(
                                        ps[:], wk[(g, e)][:, dkt * 128:(dkt + 1) * 128],
                                        h_tiles[e][:, sl],
                                        start=(e == 0), stop=(e == NE - 1))
                                nc.vector.tensor_copy(k_tiles[(g, dkt)][:, sl], ps[:])
                    # Q projections (own token blocks only)
                    if "q" not in cfg.skip and tb < NQB:
                        for eg in range(NE):
                            ps = acc_ps.tile([128, 512], F32, tag="acc", name="acc")
                            for e in range(NE):
                                nc.tensor.matmul(
                                    ps[:], wq[e][:, eg * 128:(eg + 1) * 128],
                                    h_tiles[e][:, sl],
                                    start=(e == 0), stop=(e == NE - 1))
                            nc.vector.tensor_copy(q_tiles[eg][:, sl], ps[:])

            # --- Phase B2: V projection (full tokens), both groups ---
            _mark(nc, "B2:kv")
            with ExitStack() as pkv:
                wv_pool = pkv.enter_context(tc.tile_pool(name="wvp", bufs=2))
                kv_ps = pkv.enter_context(tc.tile_pool(name="kvps", bufs=4, space="PSUM"))
                for g in (() if "kv" in cfg.skip else range(G)):
                    wv = []
                    for e in range(NE):
                        t = wv_pool.tile([128, HPG * D], F16, tag=f"wv{e}", name="wv")
                        col0 = 2 * E + g * HPG * D
                        nc.scalar.dma_start(
                            t[:], qkvT.ap()[e * 128:(e + 1) * 128, col0:col0 + HPG * D])
                        wv.append(t)
                    # V: [kpos, head, 65] tiles; col 64 = ones (denominator trick)
                    for tk in range(NTK):
                        vt = v_pool.tile([128, HPG, 65], F16, tag="vt", name="vt")
                        v_tiles[(g, tk)] = vt
                        nc.vector.memset(vt[:, :, 64:65], 1.0)
                        ps = kv_ps.tile([128, HPG * D], F32, tag="kvacc", name="kvacc")
                        off = tk * 128
                        for e in range(NE):
                            nc.tensor.matmul(ps[:], h_tiles[e][:, off:off + 128],
                                             wv[e][:],
                                             start=(e == 0), stop=(e == NE - 1))
                        nc.vector.tensor_copy(
                            vt[:, :, 0:64], ps[:].rearrange("p (h dd) -> p h dd", dd=D))

            hstk.close()  # h freed; attention does not need it

            # --- Phase C: attention, per group ---
            _mark(nc, "C:att")
            av_pool = ctx.enter_context(tc.tile_pool(name="avp", bufs=NE, side="right"))
            av_tiles = [av_pool.tile([128, T_OWN], F16, tag="avt", name="avt")
                        for _ in range(NE)]
            # prefetch fc weights + residual x while attention runs (DMA idle)
            xo_pool = ctx.enter_context(tc.tile_pool(name="xop", bufs=NE, side="right"))
            wf_pool = ctx.enter_context(tc.tile_pool(name="wfp", bufs=NE, side="right"))
            xo = []
            for e in range(NE):
                t = xo_pool.tile([128, T_OWN], F16, tag="xo", name="xo")
                nc.sync.dma_start(t[:], xT.ap()[e * 128:(e + 1) * 128, 0:T_OWN])
                xo.append(t)
            wf = []
            for e in range(NE):
                t = wf_pool.tile([128, E], F16, tag="wf", name="wf")
                nc.scalar.dma_start(t[:], fcT.ap()[e * 128:(e + 1) * 128, :])
                wf.append(t)
            if "att" in cfg.skip:
                for t in av_tiles:
                    nc.vector.memset(t[:, 0:1], 0.0)
            for g in (() if "att" in cfg.skip else range(G)):
                with ExitStack() as pa:
                    sc_ps = pa.enter_context(
                        tc.tile_pool(name=f"scps{g}", bufs=2, space="PSUM"))
                    av_ps = pa.enter_context(
                        tc.tile_pool(name=f"avps{g}", bufs=2, space="PSUM"))
                    ex_pool = pa.enter_context(tc.tile_pool(name=f"exp{g}", bufs=6))
                    rec_pool = pa.enter_context(tc.tile_pool(name=f"rec{g}", bufs=6))
                    for hp in range(NP_G):
                        hpg = g * NP_G + hp
                        av_a = av_ps.tile([65, T_OWN], F32, tag="av", name="av")
                        av_b = av_ps.tile([65, T_OWN], F32, tag="av", name="av")
                        kt = k_tiles[(g, hp)]
                        for tk in range(NTK):
                            ksl = slice(tk * 128, (tk + 1) * 128)
                            sc_a = sc_ps.tile([128, T_OWN], F32, tag="sc", name="sc")
                            sc_b = sc_ps.tile([128, T_OWN], F32, tag="sc", name="sc")
                            for qh in range(NQB):
                                qsl = slice(qh * 512, (qh + 1) * 512)
                                # rows 0-63 and 64-127 are disjoint row-groups:
                                # adjacent matmuls run concurrently on the PE
                                nc.tensor.matmul(sc_a[:, qsl], kt[0:64, ksl],
                                                 q_tiles[hpg][0:64, qsl],
                                                 start=True, stop=True)
                                nc.tensor.matmul(sc_b[:, qsl], kt[64:128, ksl],
                                                 q_tiles[hpg][64:128, qsl],
                                                 start=True, stop=True)
                            ex_a = ex_pool.tile([128, T_OWN], F16, tag="ex", name="ex")
                            ex_b = ex_pool.tile([128, T_OWN], F16, tag="ex", name="ex")
                            if "expcopy" in cfg.skip:
                                nc.vector.tensor_copy(ex_a[:], sc_a[:])
                                nc.vector.tensor_copy(ex_b[:], sc_b[:])
                            elif "exp512" in cfg.skip:
                                for qh in range(NQB):
                                    qsl = slice(qh * 512, (qh + 1) * 512)
                                    nc.scalar.activation(ex_a[:, qsl], sc_a[:, qsl], AF.Exp)
                                    nc.scalar.activation(ex_b[:, qsl], sc_b[:, qsl], AF.Exp)
                            else:
                                nc.scalar.activation(ex_a[:], sc_a[:], AF.Exp)
                                nc.scalar.activation(ex_b[:], sc_b[:], AF.Exp)
                            for head, ex_t, av_t in ((0, ex_a, av_a), (1, ex_b, av_b)):
                                vslc = v_tiles[(g, tk)][:, 2 * hp + head, :]
                                for qh in range(NQB):
                                    qsl = slice(qh * 512, (qh + 1) * 512)
                                    nc.tensor.matmul(av_t[:, qsl], vslc, ex_t[:, qsl],
                                                     start=(tk == 0), stop=(tk == NTK - 1))
                        for head, av_t in ((0, av_a), (1, av_b)):
                            rrow = rec_pool.tile([1, T_OWN], F32R, tag="rr", name="rr")
                            nc.vector.reciprocal(rrow[:], av_t[64:65, :])
                            rm = sc_ps.tile([64, T_OWN], F32, tag="sc", name="rm")
                            for qh in range(NQB):
                                qsl = slice(qh * 512, (qh + 1) * 512)
                                nc.tensor.matmul(rm[:, qsl], ones64[:],
                                                 rrow[0:1, qsl],
                                                 start=True, stop=True)
                            rms = rec_pool.tile([64, T_OWN], F32, tag="rms", name="rms")
                            nc.vector.tensor_copy(rms[:], rm[:])
                            nc.vector.tensor_tensor(
                                av_tiles[hpg][head * 64:(head + 1) * 64, :],
                                av_t[0:64, :], rms[:], OP.mult)
        # h/q/k/v freed here

        # ================= Phase 4: fc_out + residual =================
        _mark(nc, "D:fc")
        x2_pool = ctx.enter_context(tc.tile_pool(name="x2p", bufs=NE))
        x2_tiles = [x2_pool.tile([128, T_OWN], F16, tag="x2t", name="x2t")
                    for _ in range(NE)]
        with ExitStack() as p4:
            fc_ps = p4.enter_context(tc.tile_pool(name="fcps", bufs=6, space="PSUM"))
            if "fc" in cfg.skip:
                for t in x2_tiles:
                    nc.vector.memset(t[:, 0:1], 0.0)
            for og in (() if "fc" in cfg.skip else range(NE // 2)):
                ps = {(j, qh): fc_ps.tile([128, 512], F32, tag="fc", name="fc")
                      for j in range(2) for qh in range(NQB)}
                for e in range(NE):
                    for j in range(2):
                        o = og * 2 + j
                        for qh in range(NQB):
                            nc.tensor.matmul(
                                ps[(j, qh)][:], wf[e][:, o * 128:(o + 1) * 128],
                                av_tiles[e][:, qh * 512:(qh + 1) * 512],
                                start=(e == 0), stop=(e == NE - 1))
                for j in range(2):
                    o = og * 2 + j
                    for qh in range(NQB):
                        qsl = slice(qh * 512, (qh + 1) * 512)
                        nc.vector.scalar_tensor_tensor(
                            x2_tiles[o][:, qsl], ps[(j, qh)][:], fcb_t[:, o:o + 1],
                            xo[o][:, qsl], op0=OP.add, op1=OP.add)

        # ================= Phase 5: LN2 =================
        _mark(nc, "E:ln2")
        h2_pool = ctx.enter_context(tc.tile_pool(name="h2p", bufs=NE))
        h2_tiles = [h2_pool.tile([128, T_OWN], F16, tag="h2t", name="h2t")
                    for _ in range(NE)]
        with ExitStack() as p5:
            sq_pool = p5.enter_context(tc.tile_pool(name="sq2p", bufs=2))
            st_ps = p5.enter_context(tc.tile_pool(name="st2ps", bufs=2, space="PSUM"))
            row_pool = p5.enter_context(tc.tile_pool(name="rows2", bufs=2))
            map_ps = p5.enter_context(tc.tile_pool(name="map2ps", bufs=2, space="PSUM"))
            stat2_pool = p5.enter_context(tc.tile_pool(name="stat2p", bufs=1))
            srst2 = stat2_pool.tile([1, T_OWN], F32R, tag="srst2")
            snb2 = stat2_pool.tile([2, T_OWN], F32R, tag="snb2")
            nc.sync.dma_start(snb2[1:2, :], ones.ap()[0:T_OWN].unsqueeze(0).bitcast(F32R))
            _ln_stats(nc, cfg, (sq_pool, st_ps, row_pool),
                      lambda e, tb: x2_tiles[e][:, tb * 512:(tb + 1) * 512],
                      NQB, ones_col, eps_t, srst2, snb2)
            for tb in range(NQB):
                sl = slice(tb * 512, (tb + 1) * 512)
                for e in range(NE):
                    _ln_apply(nc, map_ps, ln2t[:, e * 128:(e + 1) * 128],
                              srst2, snb2, x2_tiles[e][:, sl], h2_tiles[e][:, sl], sl)

        # ================= Phase 6: MLP =================
        _mark(nc, "F:mlp1")
        g_pool = ctx.enter_context(tc.tile_pool(name="gp", bufs=NM))
        g_tiles = [g_pool.tile([128, T_OWN], F16, tag="gt", name="gt")
                   for _ in range(NM)]
        with ExitStack() as p6:
            w1_pool = p6.enter_context(tc.tile_pool(name="w1p", bufs=3))
            m1_ps = p6.enter_context(tc.tile_pool(name="m1ps", bufs=3, space="PSUM"))
            for mb in (() if "mlp" in cfg.skip else range(NM // 4)):
                w1t = []
                for e in range(NE):
                    t = w1_pool.tile([128, 512], F16, tag=f"w1_{e}", name="w1t", bufs=2)
                    nc.scalar.dma_start(
                        t[:], w1T.ap()[e * 128:(e + 1) * 128,
                                       mb * 512:(mb + 1) * 512])
                    w1t.append(t)
                for j in range(4):
                    m = mb * 4 + j
                    ps = m1_ps.tile([128, T_OWN], F32, tag="m1", name="m1")
                    for qh in range(NQB):
                        qsl = slice(qh * 512, (qh + 1) * 512)
                        for e in range(NE):
                            nc.tensor.matmul(ps[:, qsl], w1t[e][:, j * 128:(j + 1) * 128],
                                             h2_tiles[e][:, qsl],
                                             start=(e == 0), stop=(e == NE - 1))
                    nc.scalar.activation(
                        g_tiles[m][:], ps[:],
                        AF.Gelu, bias=b1_t[:, m:m + 1], scale=1.0)

        _mark(nc, "G:mlp2")
        with ExitStack() as p6b:
            w2_pool = p6b.enter_context(tc.tile_pool(name="w2p", bufs=3))
            out_pool = p6b.enter_context(tc.tile_pool(name="op", bufs=4))
            m2_ps = p6b.enter_context(tc.tile_pool(name="m2ps", bufs=8, space="PSUM"))
            for oh in (() if "mlp" in cfg.skip else range(2)):
                csl = slice(oh * 512, (oh + 1) * 512)
                ps = {(o, qh): m2_ps.tile([128, 512], F32, tag="m2", name="m2")
                      for o in range(4) for qh in range(NQB)}
                for m4 in range(NM // 4):
                    wt = w2_pool.tile([128, 4, 512], F16, tag="w2", name="w2t")
                    nc.scalar.dma_start(
                        wt[:], w2T.ap()[m4 * 512:(m4 + 1) * 512, csl].rearrange(
                            "(a p) n -> p a n", p=128))
                    for a in range(4):
                        m = m4 * 4 + a
                        for o in range(4):
                            for qh in range(NQB):
                                nc.tensor.matmul(
                                    ps[(o, qh)][:], wt[:, a, o * 128:(o + 1) * 128],
                                    g_tiles[m][:, qh * 512:(qh + 1) * 512],
                                    start=(m == 0), stop=(m == NM - 1))
                for o in range(4):
                    of = oh * 4 + o
                    for qh in range(NQB):
                        qsl = slice(qh * 512, (qh + 1) * 512)
                        ot = out_pool.tile([128, 512], F32, tag="ot", name="ot")
                        nc.vector.scalar_tensor_tensor(
                            ot[:], ps[(o, qh)][:], b2_t[:, of:of + 1],
                            x2_tiles[of][:, qsl], op0=OP.add, op1=OP.add)
                        nc.sync.dma_start(out.ap()[of * 128:(of + 1) * 128, qsl], ot[:])


# ----------------------------------------------------------------------------
# host driver
# ----------------------------------------------------------------------------
B, S, E_FULL, H_FULL, MLP_FULL = 4, 2048, 1024, 16, 4096
_cache = {}


def _get_nc():
    if "nc" not in _cache:
        _cache["nc"] = build(Cfg())
    return _cache["nc"]


def _host_prepare(x_b, roll, qkv_w, fc_w, fc_b, ln1_g, ln1_b, ln2_g, ln2_b,
                  w1, b1, w2, b2):
    S_, E = x_b.shape
    D = E // H_FULL
    xr = np.roll(x_b, -roll, axis=0)
    qkvT = np.ascontiguousarray(qkv_w.T).astype(np.float32).copy()
    qkvT[:, :E] *= D ** -0.5
    return {
        "xT": np.ascontiguousarray(xr.T).astype(np.float16),
        "qkvT": qkvT.astype(np.float16),
        "fcT": np.ascontiguousarray(fc_w.T).astype(np.float16),
        "w1T": np.ascontiguousarray(w1.T).astype(np.float16),
        "w2T": np.ascontiguousarray(w2.T).astype(np.float16),
        "ln1": np.stack([ln1_g, ln1_b]).astype(np.float32),
        "ln2": np.stack([ln2_g, ln2_b]).astype(np.float32),
        "fcb": np.asarray(fc_b, np.float32),
        "b1": np.asarray(b1, np.float32),
        "b2": np.asarray(b2, np.float32),
        "ones": np.ones((S_,), np.float32),
    }


def kernel(x, qkv_w, fc_w, fc_b, ln1_g, ln1_b, ln2_g, ln2_b, w1, b1, w2, b2):
    from concourse.bass_utils import run_bass_kernel_spmd

    x = np.ascontiguousarray(np.asarray(x, dtype=np.float32))
    args = [np.ascontiguousarray(np.asarray(a, dtype=np.float32)) for a in
            (qkv_w, fc_w, fc_b, ln1_g, ln1_b, ln2_g, ln2_b, w1, b1, w2, b2)]
    nc = _get_nc()
    in_maps = []
    for c in range(8):
        b, hf = c // 2, c % 2
        in_maps.append(_host_prepare(x[b], hf * (S // 2), *args))
    res = run_bass_kernel_spmd(nc, in_maps, list(range(8)))
    out = np.empty((B, S, E_FULL), np.float32)
    for c in range(8):
        b, hf = c // 2, c % 2
        out[b, hf * (S // 2):(hf + 1) * (S // 2), :] = res.results[c]["out"].T
    return out


# revision 30
# speedup vs baseline: 1.0071x; 1.0071x over previous
"""Trainium2 Bass kernel for a dense transformer block, sharded over 8 NeuronCores.

Sharding: core c handles batch b=c//2 and half hf=c%2 of that batch's 2048
tokens ("own" tokens). K/V are computed for the full 2048-token batch on both
cores of a pair, so no collectives are needed.

v2: fp16 matmul path (fp32 PSUM accumulation), everything SBUF-resident (no
DRAM round-trip for h), each weight matrix streamed exactly once, exp done in
[128,1024] chunks to amortize ACT overhead.
"""

import numpy as np

from contextlib import ExitStack

import concourse.bass as bass
import concourse.bacc as bacc
import concourse.tile as tile
import concourse.mybir as mybir

F32 = mybir.dt.float32
F32R = mybir.dt.float32r
F16 = mybir.dt.float16
AF = mybir.ActivationFunctionType
OP = mybir.AluOpType

EPS = 1e-5

PHASE_MARKS = []


def _mark(nc, name):
    PHASE_MARKS.append((name, int(nc.get_next_instruction_name()[2:])))


class Cfg:
    def __init__(self, E=1024, H=16, MLP=4096, T_OWN=1024, T_FULL=2048, repeat=1,
                 skip=()):
        self.E, self.H, self.MLP = E, H, MLP
        self.T_OWN, self.T_FULL = T_OWN, T_FULL
        self.D = E // H
        self.NE = E // 128          # feature tiles
        self.NM = MLP // 128        # mlp feature tiles
        self.NQB = T_OWN // 512     # own-token 512-blocks
        self.NFB = T_FULL // 512    # full-token 512-blocks
        self.NTK = T_FULL // 128    # full-token 128-blocks (k positions)
        self.G = 2                  # head groups
        self.HPG = H // self.G      # heads per group
        self.NP_G = self.HPG // 2   # head-pairs per group
        self.repeat = repeat
        self.skip = frozenset(skip)


def build(cfg: Cfg):
    E, MLP, T_OWN, T_FULL = cfg.E, cfg.MLP, cfg.T_OWN, cfg.T_FULL

    nc = bacc.Bacc("TRN2", target_bir_lowering=False, debug=False)

    d = {}
    d["xT"] = nc.dram_tensor("xT", [E, T_FULL], F16, kind="ExternalInput")
    d["qkvT"] = nc.dram_tensor("qkvT", [E, 3 * E], F16, kind="ExternalInput")
    d["fcT"] = nc.dram_tensor("fcT", [E, E], F16, kind="ExternalInput")
    d["w1T"] = nc.dram_tensor("w1T", [E, MLP], F16, kind="ExternalInput")
    d["w2T"] = nc.dram_tensor("w2T", [MLP, E], F16, kind="ExternalInput")
    d["ln1"] = nc.dram_tensor("ln1", [2, E], F32, kind="ExternalInput")
    d["ln2"] = nc.dram_tensor("ln2", [2, E], F32, kind="ExternalInput")
    d["fcb"] = nc.dram_tensor("fcb", [E], F32, kind="ExternalInput")
    d["b1"] = nc.dram_tensor("b1", [MLP], F32, kind="ExternalInput")
    d["b2"] = nc.dram_tensor("b2", [E], F32, kind="ExternalInput")
    d["ones"] = nc.dram_tensor("ones", [T_FULL], F32, kind="ExternalInput")
    d["out"] = nc.dram_tensor("out", [E, T_OWN], F32, kind="ExternalOutput")

    PHASE_MARKS.clear()
    with tile.TileContext(nc) as tc, nc.allow_low_precision(
        reason="fp16 matmul inputs by design"
    ):
        if cfg.repeat == 1:
            _body(nc, tc, cfg, d)
        else:
            with tc.For_i(0, cfg.repeat, 1, hint_engines=(
                    mybir.EngineType.PE, mybir.EngineType.Activation,
                    mybir.EngineType.DVE, mybir.EngineType.SP)):
                _body(nc, tc, cfg, d)
    nc.compile()
    return nc


def _ln_stats(nc, cfg, pools, src_fn, nblk, ones_col, eps_t, srst, snb):
    """Column stats over the feature dim via ones-matmuls (fp16 inputs).

    src_fn(e, tb) -> [128,512] F16 AP; writes rstd into srst[0:1] (f32) and
    -mu*rstd into snb[0:1] (f32)."""
    E, NE = cfg.E, cfg.NE
    sq_pool, st_ps, row_pool = pools
    for tb in range(nblk):
        sl = slice(tb * 512, (tb + 1) * 512)
        s1 = st_ps.tile([1, 512], F32, tag="s1")
        s2 = st_ps.tile([1, 512], F32, tag="s2")
        for e in range(NE):
            src = src_fn(e, tb)
            sq = sq_pool.tile([128, 512], F16, tag="sq")
            nc.vector.tensor_tensor(sq[:], src, src, OP.mult)
            nc.tensor.matmul(s1[:], ones_col[:], src, start=(e == 0), stop=(e == NE - 1))
            nc.tensor.matmul(s2[:], ones_col[:], sq[:], start=(e == 0), stop=(e == NE - 1))
        m_row = row_pool.tile([1, 512], F32, tag="mrow")
        nc.vector.tensor_scalar_mul(m_row[:], s1[:], 1.0 / E)
        v_row = row_pool.tile([1, 512], F32, tag="vrow")
        nc.vector.tensor_scalar_mul(v_row[:], s2[:], 1.0 / E)
        msq = row_pool.tile([1, 512], F32, tag="msq")
        nc.vector.tensor_tensor(msq[:], m_row[:], m_row[:], OP.mult)
        nc.vector.tensor_tensor(v_row[:], v_row[:], msq[:], OP.subtract)
        sd = row_pool.tile([1, 512], F32, tag="sd")
        nc.scalar.activation(sd[:], v_row[:], AF.Sqrt, bias=eps_t[:], scale=1.0)
        nc.vector.reciprocal(srst[0:1, sl], sd[:])
        nc.vector.scalar_tensor_tensor(
            snb[0:1, sl], m_row[:], -1.0, srst[0:1, sl].bitcast(F32),
            op0=OP.mult, op1=OP.mult)


def _ln_apply(nc, map_ps, gb_ap, srst, snb, src_ap, dst_ap, sl):
    """dst(F16) = src * (g x rstd) + (g x (-mu*rstd) + b x 1), all [128,512].

    gb_ap: [2,128] f32 AP (rows g, b) for this feature tile."""
    a_ps = map_ps.tile([128, 512], F32, tag="amap")
    nc.tensor.matmul(a_ps[:], gb_ap[0:1, :],
                     srst[0:1, sl], start=True, stop=True)
    b_ps = map_ps.tile([128, 512], F32, tag="bmap")
    nc.tensor.matmul(b_ps[:], gb_ap[0:2, :],
                     snb[0:2, sl], start=True, stop=True)
    nc.vector.tensor_tensor(dst_ap, src_ap, a_ps[:], OP.mult)
    nc.vector.tensor_tensor(dst_ap, dst_ap, b_ps[:], OP.add)


def _body(nc, tc, cfg, d):
    E, H, MLP, D = cfg.E, cfg.H, cfg.MLP, cfg.D
    NE, NM, NQB, NFB, NTK = cfg.NE, cfg.NM, cfg.NQB, cfg.NFB, cfg.NTK
    T_OWN, T_FULL, G, HPG, NP_G = cfg.T_OWN, cfg.T_FULL, cfg.G, cfg.HPG, cfg.NP_G
    xT, qkvT, fcT, w1T, w2T = d["xT"], d["qkvT"], d["fcT"], d["w1T"], d["w2T"]
    ln1, ln2, fcb, b1, b2 = d["ln1"], d["ln2"], d["fcb"], d["b1"], d["b2"]
    ones, out = d["ones"], d["out"]

    with ExitStack() as ctx:
        consts = ctx.enter_context(tc.tile_pool(name="consts", bufs=1))

        ones_col = consts.tile([128, 1], F16)
        nc.vector.memset(ones_col[:], 1.0)
        ones64 = consts.tile([1, 64], F32R)
        nc.sync.dma_start(ones64[:], ones.ap()[0:64].unsqueeze(0).bitcast(F32R))
        eps_t = consts.tile([1, 1], F32)
        nc.vector.memset(eps_t[:], EPS)
        ln1t = consts.tile([2, E], F32R)
        nc.sync.dma_start(ln1t[:], ln1.ap().bitcast(F32R))
        ln2t = consts.tile([2, E], F32R)
        nc.sync.dma_start(ln2t[:], ln2.ap().bitcast(F32R))
        fcb_t = consts.tile([128, NE], F32)
        nc.sync.dma_start(fcb_t[:], fcb.ap().rearrange("(a p) -> p a", p=128))
        b1_t = consts.tile([128, NM], F32)
        nc.sync.dma_start(b1_t[:], b1.ap().rearrange("(a p) -> p a", p=128))
        b2_t = consts.tile([128, NE], F32)
        nc.sync.dma_start(b2_t[:], b2.ap().rearrange("(a p) -> p a", p=128))

        # ================= Phases 1-3: LN1, QKV, attention =================
        with ExitStack() as p1:
            q_pool = p1.enter_context(tc.tile_pool(name="qp", bufs=NE))
            q_tiles = [q_pool.tile([128, T_OWN], F16, tag="qt", name="qt")
                       for _ in range(NE)]
            k_pool = p1.enter_context(tc.tile_pool(name="kp", bufs=G * NP_G))
            v_pool = p1.enter_context(tc.tile_pool(name="vp", bufs=G * NTK))
            k_tiles, v_tiles = {}, {}

            # --- Phase A: fused LN1 + K/Q projections (per token block) ---
            _mark(nc, "A:ln1")
            hstk = ExitStack()
            h_pool = hstk.enter_context(tc.tile_pool(name="hp", bufs=NE))
            h_tiles = [h_pool.tile([128, T_FULL], F16, tag="ht", name="ht")
                       for _ in range(NE)]
            with ExitStack() as pA:
                xf_pool = pA.enter_context(tc.tile_pool(name="xfp", bufs=2 * NE))
                sq_pool = pA.enter_context(tc.tile_pool(name="sqp", bufs=4))
                st_ps = pA.enter_context(tc.tile_pool(name="stps", bufs=1, space="PSUM"))
                row_pool = pA.enter_context(tc.tile_pool(name="rows", bufs=2))
                map_ps = pA.enter_context(tc.tile_pool(name="mapps", bufs=1, space="PSUM"))
                stat_pool = pA.enter_context(tc.tile_pool(name="statp", bufs=1))
                wq_pool = pA.enter_context(tc.tile_pool(name="wqp", bufs=NE))
                wk_pool = pA.enter_context(tc.tile_pool(name="wkp", bufs=2 * NE))
                acc_ps = pA.enter_context(tc.tile_pool(name="accps", bufs=4, space="PSUM"))
                srst1 = stat_pool.tile([1, T_FULL], F32R, tag="srst1")
                snb1 = stat_pool.tile([2, T_FULL], F32R, tag="snb1")
                nc.sync.dma_start(snb1[1:2, :], ones.ap()[0:T_FULL].unsqueeze(0).bitcast(F32R))

                xts0 = []
                for e in range(NE):
                    t = xf_pool.tile([128, 512], F16, tag="xf", name="xf")
                    eng = nc.sync if e % 2 == 0 else nc.scalar
                    eng.dma_start(t[:], xT.ap()[e * 128:(e + 1) * 128, 0:512])
                    xts0.append(t)
                wq = []
                for e in range(NE):
                    t = wq_pool.tile([128, E], F16, tag="wq", name="wq")
                    nc.scalar.dma_start(t[:], qkvT.ap()[e * 128:(e + 1) * 128, 0:E])
                    wq.append(t)
                wk = {}
                for g in range(G):
                    for e in range(NE):
                        t = wk_pool.tile([128, HPG * D], F16, tag="wk", name="wk")
                        col0 = E + g * HPG * D
                        nc.scalar.dma_start(
                            t[:], qkvT.ap()[e * 128:(e + 1) * 128, col0:col0 + HPG * D])
                        wk[(g, e)] = t
                for g in range(G):
                    for dkt in range(NP_G):
                        k_tiles[(g, dkt)] = k_pool.tile([128, T_FULL], F16,
                                                        tag="kt", name="kt")

                for tb in range(NFB):
                    sl = slice(tb * 512, (tb + 1) * 512)
                    if tb == 0:
                        xts = xts0
                    else:
                        xts = []
                        for e in range(NE):
                            t = xf_pool.tile([128, 512], F16, tag="xf", name="xf")
                            nc.sync.dma_start(t[:], xT.ap()[e * 128:(e + 1) * 128, sl])
                            xts.append(t)
                    _ln_stats(nc, cfg, (sq_pool, st_ps, row_pool),
                              lambda e, _tb: xts[e][:], 1, ones_col, eps_t,
                              srst1[0:1, sl], snb1[0:2, sl])
                    for e in range(NE):
                        _ln_apply(nc, map_ps, ln1t[:, e * 128:(e + 1) * 128],
                                  srst1, snb1, xts[e][:], h_tiles[e][:, sl], sl)
                    # K projections for this token block (both groups)
                    if "kv" not in cfg.skip:
                        for g in range(G):
                            for dkt in range(NP_G):
                                ps = acc_ps.tile([128, 512], F32, tag="acc", name="acc")
                                for e in range(NE):
                                    nc.tensor.matmul(
                                        ps[:], wk[(g, e)][:, dkt * 128:(dkt + 1) * 128],
                                        h_tiles[e][:, sl],
                                        start=(e == 0), stop=(e == NE - 1))
                                nc.vector.tensor_copy(k_tiles[(g, dkt)][:, sl], ps[:])
                    # Q projections (own token blocks only)
                    if "q" not in cfg.skip and tb < NQB:
                        for eg in range(NE):
                            ps = acc_ps.tile([128, 512], F32, tag="acc", name="acc")
                            for e in range(NE):
                                nc.tensor.matmul(
                                    ps[:], wq[e][:, eg * 128:(eg + 1) * 128],
                                    h_tiles[e][:, sl],
                                    start=(e == 0), stop=(e == NE - 1))
                            nc.vector.tensor_copy(q_tiles[eg][:, sl], ps[:])

            # --- Phase B2: V projection (full tokens), both groups ---
            _mark(nc, "B2:kv")
            with ExitStack() as pkv:
                wv_pool = pkv.enter_context(tc.tile_pool(name="wvp", bufs=2))
                kv_ps = pkv.enter_context(tc.tile_pool(name="kvps", bufs=4, space="PSUM"))
                for g in (() if "kv" in cfg.skip else range(G)):
                    wv = []
                    for e in range(NE):
                        t = wv_pool.tile([128, HPG * D], F16, tag=f"wv{e}", name="wv")
                        col0 = 2 * E + g * HPG * D
                        nc.scalar.dma_start(
                            t[:], qkvT.ap()[e * 128:(e + 1) * 128, col0:col0 + HPG * D])
                        wv.append(t)
                    # V: [kpos, head, 65] tiles; col 64 = ones (denominator trick)
                    for tk in range(NTK):
                        vt = v_pool.tile([128, HPG, 65], F16, tag="vt", name="vt")
                        v_tiles[(g, tk)] = vt
                        nc.vector.memset(vt[:, :, 64:65], 1.0)
                        ps = kv_ps.tile([128, HPG * D], F32, tag="kvacc", name="kvacc")
                        off = tk * 128
                        for e in range(NE):
                            nc.tensor.matmul(ps[:], h_tiles[e][:, off:off + 128],
                                             wv[e][:],
                                             start=(e == 0), stop=(e == NE - 1))
                        nc.vector.tensor_copy(
                            vt[:, :, 0:64], ps[:].rearrange("p (h dd) -> p h dd", dd=D))

            hstk.close()  # h freed; attention does not need it

            # --- Phase C: attention, per group ---
            _mark(nc, "C:att")
            av_pool = ctx.enter_context(tc.tile_pool(name="avp", bufs=NE, side="right"))
            av_tiles = [av_pool.tile([128, T_OWN], F16, tag="avt", name="avt")
                        for _ in range(NE)]
            # prefetch fc weights + residual x while attention runs (DMA idle)
            xo_pool = ctx.enter_context(tc.tile_pool(name="xop", bufs=NE, side="right"))
            wf_pool = ctx.enter_context(tc.tile_pool(name="wfp", bufs=NE, side="right"))
            xo = []
            for e in range(NE):
                t = xo_pool.tile([128, T_OWN], F16, tag="xo", name="xo")
                nc.sync.dma_start(t[:], xT.ap()[e * 128:(e + 1) * 128, 0:T_OWN])
                xo.append(t)
            wf = []
            for e in range(NE):
                t = wf_pool.tile([128, E], F16, tag="wf", name="wf")
                nc.scalar.dma_start(t[:], fcT.ap()[e * 128:(e + 1) * 128, :])
                wf.append(t)
            if "att" in cfg.skip:
                for t in av_tiles:
                    nc.vector.memset(t[:, 0:1], 0.0)
            for g in (() if "att" in cfg.skip else range(G)):
                with ExitStack() as pa:
                    sc_ps = pa.enter_context(
                        tc.tile_pool(name=f"scps{g}", bufs=2, space="PSUM"))
                    av_ps = pa.enter_context(
                        tc.tile_pool(name=f"avps{g}", bufs=2, space="PSUM"))
                    ex_pool = pa.enter_context(tc.tile_pool(name=f"exp{g}", bufs=6))
                    rec_pool = pa.enter_context(tc.tile_pool(name=f"rec{g}", bufs=6))
                    for hp in range(NP_G):
                        hpg = g * NP_G + hp
                        av_a = av_ps.tile([65, T_OWN], F32, tag="av", name="av")
                        av_b = av_ps.tile([65, T_OWN], F32, tag="av", name="av")
                        kt = k_tiles[(g, hp)]
                        for tk in range(NTK):
                            ksl = slice(tk * 128, (tk + 1) * 128)
                            sc_a = sc_ps.tile([128, T_OWN], F32, tag="sc", name="sc")
                            sc_b = sc_ps.tile([128, T_OWN], F32, tag="sc", name="sc")
                            for qh in range(NQB):
                                qsl = slice(qh * 512, (qh + 1) * 512)
                                # rows 0-63 and 64-127 are disjoint row-groups:
                                # adjacent matmuls run concurrently on the PE
                                nc.tensor.matmul(sc_a[:, qsl], kt[0:64, ksl],
                                                 q_tiles[hpg][0:64, qsl],
                                                 start=True, stop=True)
                                nc.tensor.matmul(sc_b[:, qsl], kt[64:128, ksl],
                                                 q_tiles[hpg][64:128, qsl],
                                                 start=True, stop=True)
                            ex_a = ex_pool.tile([128, T_OWN], F16, tag="ex", name="ex")
                            ex_b = ex_pool.tile([128, T_OWN], F16, tag="ex", name="ex")
                            if "expcopy" in cfg.skip:
                                nc.vector.tensor_copy(ex_a[:], sc_a[:])
                                nc.vector.tensor_copy(ex_b[:], sc_b[:])
                            elif "exp512" in cfg.skip:
                                for qh in range(NQB):
                                    qsl = slice(qh * 512, (qh + 1) * 512)
                                    nc.scalar.activation(ex_a[:, qsl], sc_a[:, qsl], AF.Exp)
                                    nc.scalar.activation(ex_b[:, qsl], sc_b[:, qsl], AF.Exp)
                            else:
                                nc.scalar.activation(ex_a[:], sc_a[:], AF.Exp)
                                nc.scalar.activation(ex_b[:], sc_b[:], AF.Exp)
                            for head, ex_t, av_t in ((0, ex_a, av_a), (1, ex_b, av_b)):
                                vslc = v_tiles[(g, tk)][:, 2 * hp + head, :]
                                for qh in range(NQB):
                                    qsl = slice(qh * 512, (qh + 1) * 512)
                                    nc.tensor.matmul(av_t[:, qsl], vslc, ex_t[:, qsl],
                                                     start=(tk == 0), stop=(tk == NTK - 1))
                        for head, av_t in ((0, av_a), (1, av_b)):
                            rrow = rec_pool.tile([1, T_OWN], F32R, tag="rr", name="rr")
                            nc.vector.reciprocal(rrow[:], av_t[64:65, :])
                            rm = sc_ps.tile([64, T_OWN], F32, tag="sc", name="rm")
                            for qh in range(NQB):
                                qsl = slice(qh * 512, (qh + 1) * 512)
                                nc.tensor.matmul(rm[:, qsl], ones64[:],
                                                 rrow[0:1, qsl],
                                                 start=True, stop=True)
                            rms = rec_pool.tile([64, T_OWN], F32, tag="rms", name="rms")
                            nc.vector.tensor_copy(rms[:], rm[:])
                            nc.vector.tensor_tensor(
                                av_tiles[hpg][head * 64:(head + 1) * 64, :],
                                av_t[0:64, :], rms[:], OP.mult)
        # h/q/k/v freed here

        # ================= Phase 4: fc_out + residual =================
        _mark(nc, "D:fc")
        x2_pool = ctx.enter_context(tc.tile_pool(name="x2p", bufs=NE))
        x2_tiles = [x2_pool.tile([128, T_OWN], F16, tag="x2t", name="x2t")
                    for _ in range(NE)]
        with ExitStack() as p4:
            fc_ps = p4.enter_context(tc.tile_pool(name="fcps", bufs=6, space="PSUM"))
            if "fc" in cfg.skip:
                for t in x2_tiles:
                    nc.vector.memset(t[:, 0:1], 0.0)
            for og in (() if "fc" in cfg.skip else range(NE // 2)):
                ps = {(j, qh): fc_ps.tile([128, 512], F32, tag="fc", name="fc")
                      for j in range(2) for qh in range(NQB)}
                for e in range(NE):
                    for j in range(2):
                        o = og * 2 + j
                        for qh in range(NQB):
                            nc.tensor.matmul(
                                ps[(j, qh)][:], wf[e][:, o * 128:(o + 1) * 128],
                                av_tiles[e][:, qh * 512:(qh + 1) * 512],
                                start=(e == 0), stop=(e == NE - 1))
                for j in range(2):
                    o = og * 2 + j
                    for qh in range(NQB):
                        qsl = slice(qh * 512, (qh + 1) * 512)
                        nc.vector.scalar_tensor_tensor(
                            x2_tiles[o][:, qsl], ps[(j, qh)][:], fcb_t[:, o:o + 1],
                            xo[o][:, qsl], op0=OP.add, op1=OP.add)

        # ================= Phase 5: LN2 =================
        _mark(nc, "E:ln2")
        h2_pool = ctx.enter_context(tc.tile_pool(name="h2p", bufs=NE))
        h2_tiles = [h2_pool.tile([128, T_OWN], F16, tag="h2t", name="h2t")
                    for _ in range(NE)]
        with ExitStack() as p5:
            sq_pool = p5.enter_context(tc.tile_pool(name="sq2p", bufs=2))
            st_ps = p5.enter_context(tc.tile_pool(name="st2ps", bufs=2, space="PSUM"))
            row_pool = p5.enter_context(tc.tile_pool(name="rows2", bufs=2))
            map_ps = p5.enter_context(tc.tile_pool(name="map2ps", bufs=2, space="PSUM"))
            stat2_pool = p5.enter_context(tc.tile_pool(name="stat2p", bufs=1))
            srst2 = stat2_pool.tile([1, T_OWN], F32R, tag="srst2")
            snb2 = stat2_pool.tile([2, T_OWN], F32R, tag="snb2")
            nc.sync.dma_start(snb2[1:2, :], ones.ap()[0:T_OWN].unsqueeze(0).bitcast(F32R))
            _ln_stats(nc, cfg, (sq_pool, st_ps, row_pool),
                      lambda e, tb: x2_tiles[e][:, tb * 512:(tb + 1) * 512],
                      NQB, ones_col, eps_t, srst2, snb2)
            for tb in range(NQB):
                sl = slice(tb * 512, (tb + 1) * 512)
                for e in range(NE):
                    _ln_apply(nc, map_ps, ln2t[:, e * 128:(e + 1) * 128],
                              srst2, snb2, x2_tiles[e][:, sl], h2_tiles[e][:, sl], sl)

        # ================= Phase 6: MLP =================
        _mark(nc, "F:mlp1")
        g_pool = ctx.enter_context(tc.tile_pool(name="gp", bufs=NM))
        g_tiles = [g_pool.tile([128, T_OWN], F16, tag="gt", name="gt")
                   for _ in range(NM)]
        with ExitStack() as p6:
            w1_pool = p6.enter_context(tc.tile_pool(name="w1p", bufs=3))
            m1_ps = p6.enter_context(tc.tile_pool(name="m1ps", bufs=3, space="PSUM"))
            for mb in (() if "mlp" in cfg.skip else range(NM // 4)):
                w1b = w1_pool.tile([128, NE, 512], F16, tag="w1b", name="w1b", bufs=2)
                nc.scalar.dma_start(
                    w1b[:], w1T.ap()[:, mb * 512:(mb + 1) * 512].rearrange(
                        "(a p) n -> p a n", p=128))
                w1t = [w1b[:, e, :] for e in range(NE)]
                for j in range(4):
                    m = mb * 4 + j
                    ps = m1_ps.tile([128, T_OWN], F32, tag="m1", name="m1")
                    for qh in range(NQB):
                        qsl = slice(qh * 512, (qh + 1) * 512)
                        for e in range(NE):
                            nc.tensor.matmul(ps[:, qsl], w1t[e][:, j * 128:(j + 1) * 128],
                                             h2_tiles[e][:, qsl],
                                             start=(e == 0), stop=(e == NE - 1))
                    nc.scalar.activation(
                        g_tiles[m][:], ps[:],
                        AF.Gelu, bias=b1_t[:, m:m + 1], scale=1.0)

        _mark(nc, "G:mlp2")
        with ExitStack() as p6b:
            w2_pool = p6b.enter_context(tc.tile_pool(name="w2p", bufs=3))
            out_pool = p6b.enter_context(tc.tile_pool(name="op", bufs=4))
            m2_ps = p6b.enter_context(tc.tile_pool(name="m2ps", bufs=8, space="PSUM"))
            for oh in (() if "mlp" in cfg.skip else range(2)):
                csl = slice(oh * 512, (oh + 1) * 512)
                ps = {(o, qh): m2_ps.tile([128, 512], F32, tag="m2", name="m2")
                      for o in range(4) for qh in range(NQB)}
                for m4 in range(NM // 4):
                    wt = w2_pool.tile([128, 4, 512], F16, tag="w2", name="w2t")
                    nc.scalar.dma_start(
                        wt[:], w2T.ap()[m4 * 512:(m4 + 1) * 512, csl].rearrange(
                            "(a p) n -> p a n", p=128))
                    for a in range(4):
                        m = m4 * 4 + a
                        for o in range(4):
                            for qh in range(NQB):
                                nc.tensor.matmul(
                                    ps[(o, qh)][:], wt[:, a, o * 128:(o + 1) * 128],
                                    g_tiles[m][:, qh * 512:(qh + 1) * 512],
                                    start=(m == 0), stop=(m == NM - 1))
                for o in range(4):
                    of = oh * 4 + o
                    for qh in range(NQB):
                        qsl = slice(qh * 512, (qh + 1) * 512)
                        ot = out_pool.tile([128, 512], F32, tag="ot", name="ot")
                        nc.vector.scalar_tensor_tensor(
                            ot[:], ps[(o, qh)][:], b2_t[:, of:of + 1],
                            x2_tiles[of][:, qsl], op0=OP.add, op1=OP.add)
                        nc.sync.dma_start(out.ap()[of * 128:(of + 1) * 128, qsl], ot[:])


# ----------------------------------------------------------------------------
# host driver
# ----------------------------------------------------------------------------
B, S, E_FULL, H_FULL, MLP_FULL = 4, 2048, 1024, 16, 4096
_cache = {}


def _get_nc():
    if "nc" not in _cache:
        _cache["nc"] = build(Cfg())
    return _cache["nc"]


def _host_prepare(x_b, roll, qkv_w, fc_w, fc_b, ln1_g, ln1_b, ln2_g, ln2_b,
                  w1, b1, w2, b2):
    S_, E = x_b.shape
    D = E // H_FULL
    xr = np.roll(x_b, -roll, axis=0)
    qkvT = np.ascontiguousarray(qkv_w.T).astype(np.float32).copy()
    qkvT[:, :E] *= D ** -0.5
    return {
        "xT": np.ascontiguousarray(xr.T).astype(np.float16),
        "qkvT": qkvT.astype(np.float16),
        "fcT": np.ascontiguousarray(fc_w.T).astype(np.float16),
        "w1T": np.ascontiguousarray(w1.T).astype(np.float16),
        "w2T": np.ascontiguousarray(w2.T).astype(np.float16),
        "ln1": np.stack([ln1_g, ln1_b]).astype(np.float32),
        "ln2": np.stack([ln2_g, ln2_b]).astype(np.float32),
        "fcb": np.asarray(fc_b, np.float32),
        "b1": np.asarray(b1, np.float32),
        "b2": np.asarray(b2, np.float32),
        "ones": np.ones((S_,), np.float32),
    }


def kernel(x, qkv_w, fc_w, fc_b, ln1_g, ln1_b, ln2_g, ln2_b, w1, b1, w2, b2):
    from concourse.bass_utils import run_bass_kernel_spmd

    x = np.ascontiguousarray(np.asarray(x, dtype=np.float32))
    args = [np.ascontiguousarray(np.asarray(a, dtype=np.float32)) for a in
            (qkv_w, fc_w, fc_b, ln1_g, ln1_b, ln2_g, ln2_b, w1, b1, w2, b2)]
    nc = _get_nc()
    in_maps = []
    for c in range(8):
        b, hf = c // 2, c % 2
        in_maps.append(_host_prepare(x[b], hf * (S // 2), *args))
    res = run_bass_kernel_spmd(nc, in_maps, list(range(8)))
    out = np.empty((B, S, E_FULL), np.float32)
    for c in range(8):
        b, hf = c // 2, c % 2
        out[b, hf * (S // 2):(hf + 1) * (S // 2), :] = res.results[c]["out"].T
    return out
